# revision 8
# baseline (speedup 1.0000x reference)
"""Trainium2 Bass kernel for nn_Euclid_FC: out[b,o] = -0.5 * ||x[b,:] - W[:,o]||^2.

Computed as x@W - 0.5*||x_b||^2 - 0.5*||w_o||^2. The device runs the
2048x1024x4096 GEMM (99.85% of the FLOPs) in fp8-e4m3 DoubleRow; the two
rank-1 bias terms are precomputed on the host during sharding (as the
baseline already did) and folded in f32 during the host-side gather.

Sharding (8 cores): 2-way over batch x 4-way over the output dim. Each core
computes a [1024, 1024] output block from x^T slice [1024, 1024] and W slice
[1024, 1024] (the traffic-minimal split).

Device schedule per core (v5). Trace findings this is built around: the PE
clock runs at 50% until a fixed HAM grant at ~16.3us (every us of earlier
GEMM start saves 0.5us), input DMA is queue/HBM-capped at ~270GB/s so one
packed queue in arrival order beats two parallel queues, DMA completion
semaphores land ~1.3us after the wire, and there is a ~9us fixed teardown.
  - x^T and W ship packed in one pi-major tensor; all input DMAs ride the
    sync queue in exactly the order the GEMM consumes them: the K-chunk-0
    pieces wave A needs first (x^T bands 0-3, W n=0, W n=1; 128KB each so
    the first DR matmul starts ~1.5us earlier), then packed chunks 1-3,
    then chunk-0's x^T bands 4-7 (only needed by wave B at ~20us);
  - 4 PE warmup matmuls run during the DMA head;
  - wave A (bands 0-3, all 8 PSUM banks) runs K-chunk-outer, streaming over
    arriving chunks; wave B (bands 4-7) runs K-inner once input is resident;
  - output is fp8-e4m3 (|x@W| < 200 << 448; rank-1 terms are added on host
    in f32, keeping rel err ~1.4e-3): halves writeback bytes;
  - the epilogue is a bare PSUM -> SBUF fp8 copy per half-band (vector n=0,
    scalar n=1), each half DMA'd immediately (gpsimd n=0 / scalar n=1
    queues, leaving the sync queue to input);
  - a final fp8 matmul reading the last band keeps the PE awake through
    writeback.
"""

import sys

if "/opt/trn_rl_repo" not in sys.path:
    sys.path.insert(0, "/opt/trn_rl_repo")

import ml_dtypes
import numpy as np

BATCH, D_IN, D_OUT = 2048, 1024, 4096
N_CORES = 8
R, C = 2, 4  # batch split x out-dim split
BB = BATCH // R  # 1024 batch rows per core
OO = D_OUT // C  # 1024 out cols per core
KT = D_IN // 128  # 8 K-subtiles
P = 128

_cached = {}


def _round_fp32r(a):
    """Round fp32 array to fp32r (11 explicit mantissa bits), RTNE."""
    b = np.ascontiguousarray(a, dtype=np.float32).view(np.uint32).copy()
    bias = ((b >> 12) & 1) + 0x7FF
    b += bias
    b &= np.uint32(0xFFFFF000)
    return b.view(np.float32)


def _build_program(mm_dtype_name="float8e4", out_dtype_name="float8e4"):
    import concourse.mybir as mybir
    import concourse.tile as tile
    from concourse import bacc

    f32 = mybir.dt.float32
    mm_dt = getattr(mybir.dt, mm_dtype_name)
    out_dt = getattr(mybir.dt, out_dtype_name)
    assert mm_dtype_name == "float8e4", "v5 schedule is fp8-DR only"

    nc = bacc.Bacc("TRN2", target_bir_lowering=False, debug=False, num_devices=N_CORES)
    # pi-major layout [partition, K-subtile, free], x^T and W packed along
    # the free dim (x^T at 0:1024, W at 1024:2048).
    xw_d = nc.dram_tensor("xw", [P, KT, BB + OO], mm_dt, kind="ExternalInput").ap()
    out_d = nc.dram_tensor("out", [BB, OO], out_dt, kind="ExternalOutput").ap()

    copy_fn = mybir.ActivationFunctionType.Copy
    DR = mybir.MatmulPerfMode.DoubleRow

    N_TILES = OO // 512  # 2
    N_CHUNKS = 4  # K chunks of 2 subtiles each (one DR matmul per chunk)
    WAVE_A = list(range(4))  # bands 0-3: chunk-outer
    WAVE_B = list(range(4, 8))  # bands 4-7: K-inner

    with tile.TileContext(nc) as tc:
        with (
            tc.tile_pool(name="ops", bufs=1) as opool,
            tc.tile_pool(name="otp", bufs=8) as otpool,
            tc.tile_pool(name="ps", bufs=8, space="PSUM") as pspool,
        ):
            # --- input DMAs, all on the sync queue, in consumption order ---
            xtA = opool.tile([P, 2, 512], mm_dt, tag="xtA", name="xtA")
            nc.sync.dma_start(xtA[:], xw_d[:, 0:2, 0:512])
            wA = opool.tile([P, 2, 512], mm_dt, tag="wA", name="wA")
            nc.sync.dma_start(wA[:], xw_d[:, 0:2, BB : BB + 512])
            wB = opool.tile([P, 2, 512], mm_dt, tag="wB", name="wB")
            nc.sync.dma_start(wB[:], xw_d[:, 0:2, BB + 512 : BB + 1024])
            xw_sb = {}
            for c in range(1, N_CHUNKS):
                t = opool.tile([P, 2, BB + OO], mm_dt, tag=f"xw{c}", name=f"xw{c}")
                nc.sync.dma_start(t[:], xw_d[:, 2 * c : 2 * c + 2, :])
                xw_sb[c] = t
            # chunk-0 x^T for bands 4-7: wave B doesn't touch it until ~20us
            xtB = opool.tile([P, 2, 512], mm_dt, tag="xtB", name="xtB")
            nc.sync.dma_start(xtB[:], xw_d[:, 0:2, 512:1024])

            warm_in = opool.tile([P, 512], mm_dt, tag="warm")
            nc.vector.memset(warm_in[:], 0)

            # --- tensor: dense warmups (only dep: warm memset); these also
            # accrue the PE-busy credit that moves the HAM full-clock grant
            # earlier (~6.2us of busy time required) ---
            def warmup(k):
                for _ in range(k):
                    warm_ps = pspool.tile([P, 512], f32, tag="ps")
                    nc.tensor.matmul(
                        warm_ps[:],
                        lhsT=warm_in[:, :P],
                        rhs=warm_in[:],
                        start=True,
                        stop=True,
                    )

            warmup(6)

            def mm(ps, c, m, n, start, stop):
                if c == 0:
                    lhsT = (xtA if m < 4 else xtB)[:, :, (m % 4) * P : (m % 4 + 1) * P]
                    rhs = (wA if n == 0 else wB)[:, :, 0:512]
                else:
                    t = xw_sb[c]
                    lhsT = t[:, :, m * P : (m + 1) * P]
                    rhs = t[:, :, BB + n * 512 : BB + (n + 1) * 512]
                nc.tensor.matmul(
                    ps[:], lhsT=lhsT, rhs=rhs, start=start, stop=stop, perf_mode=DR
                )

            def epilogue(ot, m, ps0, ps1):
                # PSUM -> SBUF fp8 halves in parallel (vector / scalar), each
                # half DMA'd out immediately on its own queue.
                nc.vector.tensor_copy(out=ot[:, 0:512], in_=ps0[:])
                nc.gpsimd.dma_start(out_d[m * P : (m + 1) * P, 0:512], ot[:, 0:512])
                nc.scalar.activation(
                    out=ot[:, 512:1024], in_=ps1[:], func=copy_fn, bias=0.0
                )
                nc.scalar.dma_start(out_d[m * P : (m + 1) * P, 512:1024], ot[:, 512:1024])

            ot_tiles = {}

            # --- wave A: bands 0-3, chunk-outer (stream over arriving DMAs) ---
            ps_a = {}
            for m in WAVE_A:
                ot_tiles[m] = otpool.tile([P, OO], out_dt, tag="ot", name=f"ot{m}")
                for n in range(N_TILES):
                    ps_a[(m, n)] = pspool.tile(
                        [P, 512], f32, tag="ps", name=f"psA{m}_{n}"
                    )
            for c in range(N_CHUNKS):
                last_c = c == N_CHUNKS - 1
                for n in range(N_TILES):
                    for m in WAVE_A:
                        mm(ps_a[(m, n)], c, m, n, start=(c == 0), stop=last_c)
                if c == 0:
                    warmup(2)
                if last_c:
                    for m in WAVE_A:
                        epilogue(ot_tiles[m], m, ps_a[(m, 0)], ps_a[(m, 1)])

            # --- wave B: bands 4-7, K-inner per tile (input fully resident) ---
            for m in WAVE_B:
                ot = otpool.tile([P, OO], out_dt, tag="ot", name=f"ot{m}")
                ot_tiles[m] = ot
                ps_b = []
                for n in range(N_TILES):
                    ps = pspool.tile([P, 512], f32, tag="ps", name=f"psB{m}_{n}")
                    for c in range(N_CHUNKS):
                        mm(ps, c, m, n, start=(c == 0), stop=(c == N_CHUNKS - 1))
                    ps_b.append(ps)
                epilogue(ot, m, ps_b[0], ps_b[1])

            # keep the PE awake through writeback: a dummy fp8 matmul that
            # reads the last band (dep on its final epilogue copies).
            ot7 = ot_tiles[WAVE_B[-1]]
            keep_ps = pspool.tile([P, 512], f32, tag="ps")
            nc.tensor.matmul(
                keep_ps[:],
                lhsT=ot7[:, 896:1024],
                rhs=ot7[:, 512:1024],
                start=True,
                stop=True,
            )
    nc.compile()
    return nc


def _to_mm(a, mm_dtype_name):
    if mm_dtype_name == "bfloat16":
        return a.astype(ml_dtypes.bfloat16)
    if mm_dtype_name == "float8e4":
        return a.astype(ml_dtypes.float8_e4m3)
    if mm_dtype_name == "float32r":
        return _round_fp32r(a)
    return a.astype(np.float32)


def _shard_inputs(x, W, mm_dtype_name):
    """Per-core in_maps (packed pi-major x^T / W) + host-side bias vectors."""
    x = np.asarray(x, dtype=np.float32)
    W = np.asarray(W, dtype=np.float32)
    xsqh = -0.5 * np.einsum("bi,bi->b", x.astype(np.float64), x.astype(np.float64))
    wsqh = -0.5 * np.einsum("io,io->o", W.astype(np.float64), W.astype(np.float64))

    def pi_major(a2d, free):
        """[K, free] -> [P, KT, free] (partition-major)."""
        return np.ascontiguousarray(a2d.reshape(KT, P, free).transpose(1, 0, 2))

    xt_shards = []
    for i in range(R):
        xs = x[i * BB : (i + 1) * BB]
        xt_shards.append(pi_major(_to_mm(np.ascontiguousarray(xs.T), mm_dtype_name), BB))

    w_shards = []
    for j in range(C):
        w_shards.append(pi_major(_to_mm(W[:, j * OO : (j + 1) * OO], mm_dtype_name), OO))

    # pack x^T and W along the free dim: [P, KT, BB + OO]
    xw_shards = {}
    for core in range(N_CORES):
        i, j = divmod(core, C)
        if (i, j) not in xw_shards:
            xw_shards[(i, j)] = np.ascontiguousarray(
                np.concatenate([xt_shards[i], w_shards[j]], axis=2)
            )

    in_maps = []
    for core in range(N_CORES):
        i, j = divmod(core, C)
        in_maps.append({"xw": xw_shards[(i, j)]})
    return in_maps, xsqh, wsqh


def _gather(results, xsqh, wsqh):
    out = np.empty((BATCH, D_OUT), dtype=np.float32)
    for core in range(N_CORES):
        i, j = divmod(core, C)
        out[i * BB : (i + 1) * BB, j * OO : (j + 1) * OO] = results[core][
            "out"
        ].astype(np.float32)
    # fold the rank-1 bias terms: out = x@W - 0.5||x_b||^2 - 0.5||w_o||^2
    out += xsqh.astype(np.float32)[:, None]
    out += wsqh.astype(np.float32)[None, :]
    return out


def run(x, W, trace=False, mm_dtype_name="float8e4", out16=True):
    from concourse import bass_utils

    key = mm_dtype_name
    if key not in _cached:
        _cached[key] = _build_program(mm_dtype_name)
    nc = _cached[key]
    in_maps, xsqh, wsqh = _shard_inputs(x, W, mm_dtype_name)
    res = bass_utils.run_bass_kernel_spmd(
        nc, in_maps, core_ids=list(range(N_CORES)), trace=trace
    )
    return _gather(res.results, xsqh, wsqh), res


def kernel(x, W):
    out, _ = run(x, W, trace=False, mm_dtype_name="float8e4")
    return out


# revision 9
# speedup vs baseline: 1.0193x; 1.0193x over previous
"""Trainium2 Bass kernel for nn_Euclid_FC: out[b,o] = -0.5 * ||x[b,:] - W[:,o]||^2.

Computed as x@W - 0.5*||x_b||^2 - 0.5*||w_o||^2. The device runs the
2048x1024x4096 GEMM (99.85% of the FLOPs) in fp8-e4m3 DoubleRow; the two
rank-1 bias terms are precomputed on the host during sharding (as the
baseline already did) and folded in f32 during the host-side gather.

Sharding (8 cores): 2-way over batch x 4-way over the output dim. Each core
computes a [1024, 1024] output block from x^T slice [1024, 1024] and W slice
[1024, 1024] (the traffic-minimal split).

Device schedule per core (v8). Trace findings this is built around: the NC
runs at 50% clock until a HAM grant that fires after ~4-6us of dense PE
activity (so dummy warmups from the earliest instant pull the grant from
~16.3us to ~12.6us); input DMA completion semaphores follow a fixed
~170->400GB/s schedule; there is a ~9us fixed teardown after the last DMA.
  - the input ships as ONE flat piece-ordered tensor [P, 16, 2, 512]:
    twelve 128KB wave-A pieces first (per K-chunk: x^T bands 0-3, W n=0,
    W n=1), then the four wave-B x^T pieces. Thirteen sync-queue DMAs in
    exactly consumption order - wave A's last dependency lands ~2us
    earlier than with monolithic 512KB chunks;
  - 6 dense PE warmup matmuls run during the DMA head (HAM credit + cover);
  - wave A (bands 0-3, all 8 PSUM banks) runs K-chunk-outer, streaming over
    arriving pieces; wave B (bands 4-7) runs K-inner once banks free;
  - output is fp8-e4m3 (|x@W| < 200 << 448; rank-1 terms are added on host
    in f32, keeping rel err ~1.4e-3): halves writeback bytes;
  - the epilogue is a bare PSUM -> SBUF fp8 copy per half-band (vector n=0,
    scalar n=1), each half DMA'd immediately (gpsimd n=0 / scalar n=1
    queues, leaving the sync queue to input);
  - a final fp8 matmul reading the last band keeps the PE awake through
    writeback.
"""

import sys

if "/opt/trn_rl_repo" not in sys.path:
    sys.path.insert(0, "/opt/trn_rl_repo")

import ml_dtypes
import numpy as np

BATCH, D_IN, D_OUT = 2048, 1024, 4096
N_CORES = 8
R, C = 2, 4  # batch split x out-dim split
BB = BATCH // R  # 1024 batch rows per core
OO = D_OUT // C  # 1024 out cols per core
KT = D_IN // 128  # 8 K-subtiles
P = 128

_cached = {}


def _round_fp32r(a):
    """Round fp32 array to fp32r (11 explicit mantissa bits), RTNE."""
    b = np.ascontiguousarray(a, dtype=np.float32).view(np.uint32).copy()
    bias = ((b >> 12) & 1) + 0x7FF
    b += bias
    b &= np.uint32(0xFFFFF000)
    return b.view(np.float32)


def _build_program(mm_dtype_name="float8e4", out_dtype_name="float8e4"):
    import concourse.mybir as mybir
    import concourse.tile as tile
    from concourse import bacc

    f32 = mybir.dt.float32
    mm_dt = getattr(mybir.dt, mm_dtype_name)
    out_dt = getattr(mybir.dt, out_dtype_name)
    assert mm_dtype_name == "float8e4", "v8 schedule is fp8-DR only"

    nc = bacc.Bacc("TRN2", target_bir_lowering=False, debug=False, num_devices=N_CORES)
    # flat piece-ordered input: 16 pieces of [2 K-subtiles, 512 free-cols],
    # contiguous per partition, DMA'd in exactly this order.
    xw_d = nc.dram_tensor("xw", [P, 16, 2, 512], mm_dt, kind="ExternalInput").ap()
    out_d = nc.dram_tensor("out", [BB, OO], out_dt, kind="ExternalOutput").ap()

    copy_fn = mybir.ActivationFunctionType.Copy
    DR = mybir.MatmulPerfMode.DoubleRow

    N_TILES = OO // 512  # 2
    N_CHUNKS = 4  # K chunks of 2 subtiles each (one DR matmul per chunk)
    WAVE_A = list(range(4))  # bands 0-3: chunk-outer
    WAVE_B = list(range(4, 8))  # bands 4-7: K-inner

    with tile.TileContext(nc) as tc:
        with (
            tc.tile_pool(name="ops", bufs=1) as opool,
            tc.tile_pool(name="otp", bufs=8) as otpool,
            tc.tile_pool(name="ps", bufs=8, space="PSUM") as pspool,
        ):
            # --- input DMAs, all on the sync queue, in consumption order:
            # per chunk c: x^T bands 0-3 (piece 3c), W n=0 (3c+1), W n=1
            # (3c+2); then the wave-B x^T pieces 12..15 as one DMA.
            xtA_sb, wA_sb, wB_sb = {}, {}, {}
            for c in range(N_CHUNKS):
                t = opool.tile([P, 2, 512], mm_dt, tag=f"xtA{c}", name=f"xtA{c}")
                nc.sync.dma_start(t[:], xw_d[:, 3 * c, :, :])
                xtA_sb[c] = t
                t = opool.tile([P, 2, 512], mm_dt, tag=f"wA{c}", name=f"wA{c}")
                nc.sync.dma_start(t[:], xw_d[:, 3 * c + 1, :, :])
                wA_sb[c] = t
                t = opool.tile([P, 2, 512], mm_dt, tag=f"wB{c}", name=f"wB{c}")
                nc.sync.dma_start(t[:], xw_d[:, 3 * c + 2, :, :])
                wB_sb[c] = t
            xtB_sb = opool.tile([P, 4, 2, 512], mm_dt, tag="xtB", name="xtB")
            nc.sync.dma_start(xtB_sb[:], xw_d[:, 12:16, :, :])

            warm_in = opool.tile([P, 512], mm_dt, tag="warm")
            nc.vector.memset(warm_in[:], 0)

            # --- tensor: dense warmups (only dep: warm memset); they accrue
            # the PE-activity credit that moves the HAM full-clock grant
            # earlier, and cover the first-piece DMA wait ---
            for _ in range(6):
                warm_ps = pspool.tile([P, 512], f32, tag="ps")
                nc.tensor.matmul(
                    warm_ps[:],
                    lhsT=warm_in[:, :P],
                    rhs=warm_in[:],
                    start=True,
                    stop=True,
                )

            def mm(ps, c, m, n, start, stop):
                if m < 4:
                    lhsT = xtA_sb[c][:, :, (m % 4) * P : (m % 4 + 1) * P]
                else:
                    lhsT = xtB_sb[:, c, :, (m % 4) * P : (m % 4 + 1) * P]
                rhs = (wA_sb[c] if n == 0 else wB_sb[c])[:, :, :]
                nc.tensor.matmul(
                    ps[:], lhsT=lhsT, rhs=rhs, start=start, stop=stop, perf_mode=DR
                )

            def epilogue(ot, m, ps0, ps1):
                # PSUM -> SBUF fp8 halves in parallel (vector / scalar), each
                # half DMA'd out immediately on its own queue.
                nc.vector.tensor_copy(out=ot[:, 0:512], in_=ps0[:])
                nc.gpsimd.dma_start(out_d[m * P : (m + 1) * P, 0:512], ot[:, 0:512])
                nc.scalar.activation(
                    out=ot[:, 512:1024], in_=ps1[:], func=copy_fn, bias=0.0
                )
                nc.scalar.dma_start(out_d[m * P : (m + 1) * P, 512:1024], ot[:, 512:1024])

            ot_tiles = {}

            # --- wave A: bands 0-3, chunk-outer (stream over arriving DMAs) ---
            ps_a = {}
            for m in WAVE_A:
                ot_tiles[m] = otpool.tile([P, OO], out_dt, tag="ot", name=f"ot{m}")
                for n in range(N_TILES):
                    ps_a[(m, n)] = pspool.tile(
                        [P, 512], f32, tag="ps", name=f"psA{m}_{n}"
                    )
            for c in range(N_CHUNKS):
                last_c = c == N_CHUNKS - 1
                for n in range(N_TILES):
                    for m in WAVE_A:
                        mm(ps_a[(m, n)], c, m, n, start=(c == 0), stop=last_c)
                if last_c:
                    for m in WAVE_A:
                        epilogue(ot_tiles[m], m, ps_a[(m, 0)], ps_a[(m, 1)])

            # --- wave B: bands 4-7, K-inner per tile (input fully resident) ---
            for m in WAVE_B:
                ot = otpool.tile([P, OO], out_dt, tag="ot", name=f"ot{m}")
                ot_tiles[m] = ot
                ps_b = []
                for n in range(N_TILES):
                    ps = pspool.tile([P, 512], f32, tag="ps", name=f"psB{m}_{n}")
                    for c in range(N_CHUNKS):
                        mm(ps, c, m, n, start=(c == 0), stop=(c == N_CHUNKS - 1))
                    ps_b.append(ps)
                epilogue(ot, m, ps_b[0], ps_b[1])

            # keep the PE awake through writeback: a dummy fp8 matmul that
            # reads the last band (dep on its final epilogue copies).
            ot7 = ot_tiles[WAVE_B[-1]]
            keep_ps = pspool.tile([P, 512], f32, tag="ps")
            nc.tensor.matmul(
                keep_ps[:],
                lhsT=ot7[:, 896:1024],
                rhs=ot7[:, 512:1024],
                start=True,
                stop=True,
            )
    nc.compile()
    return nc


def _to_mm(a, mm_dtype_name):
    if mm_dtype_name == "bfloat16":
        return a.astype(ml_dtypes.bfloat16)
    if mm_dtype_name == "float8e4":
        return a.astype(ml_dtypes.float8_e4m3)
    if mm_dtype_name == "float32r":
        return _round_fp32r(a)
    return a.astype(np.float32)


def _shard_inputs(x, W, mm_dtype_name):
    """Per-core in_maps (flat piece-ordered input) + host-side bias vectors."""
    x = np.asarray(x, dtype=np.float32)
    W = np.asarray(W, dtype=np.float32)
    xsqh = -0.5 * np.einsum("bi,bi->b", x.astype(np.float64), x.astype(np.float64))
    wsqh = -0.5 * np.einsum("io,io->o", W.astype(np.float64), W.astype(np.float64))

    def pi_major(a2d, free):
        """[K, free] -> [P, KT, free] (partition-major)."""
        return np.ascontiguousarray(a2d.reshape(KT, P, free).transpose(1, 0, 2))

    xt_shards = []
    for i in range(R):
        xs = x[i * BB : (i + 1) * BB]
        xt_shards.append(pi_major(_to_mm(np.ascontiguousarray(xs.T), mm_dtype_name), BB))

    w_shards = []
    for j in range(C):
        w_shards.append(pi_major(_to_mm(W[:, j * OO : (j + 1) * OO], mm_dtype_name), OO))

    # flat piece order [P, 16, 2, 512]: per chunk c: x^T bands 0-3 / W n=0 /
    # W n=1; then the 4 wave-B x^T pieces (bands 4-7)
    xw_shards = {}
    for core in range(N_CORES):
        i, j = divmod(core, C)
        if (i, j) not in xw_shards:
            xt, w = xt_shards[i], w_shards[j]
            pieces = []
            for c in range(4):
                pieces.append(xt[:, 2 * c : 2 * c + 2, 0:512])
                pieces.append(w[:, 2 * c : 2 * c + 2, 0:512])
                pieces.append(w[:, 2 * c : 2 * c + 2, 512:1024])
            for c in range(4):
                pieces.append(xt[:, 2 * c : 2 * c + 2, 512:1024])
            xw_shards[(i, j)] = np.ascontiguousarray(
                np.stack(pieces, axis=1)  # [P, 16, 2, 512]
            )

    in_maps = []
    for core in range(N_CORES):
        i, j = divmod(core, C)
        in_maps.append({"xw": xw_shards[(i, j)]})
    return in_maps, xsqh, wsqh


def _gather(results, xsqh, wsqh):
    out = np.empty((BATCH, D_OUT), dtype=np.float32)
    for core in range(N_CORES):
        i, j = divmod(core, C)
        out[i * BB : (i + 1) * BB, j * OO : (j + 1) * OO] = results[core][
            "out"
        ].astype(np.float32)
    # fold the rank-1 bias terms: out = x@W - 0.5||x_b||^2 - 0.5||w_o||^2
    out += xsqh.astype(np.float32)[:, None]
    out += wsqh.astype(np.float32)[None, :]
    return out


def run(x, W, trace=False, mm_dtype_name="float8e4", out16=True):
    from concourse import bass_utils

    key = mm_dtype_name
    if key not in _cached:
        _cached[key] = _build_program(mm_dtype_name)
    nc = _cached[key]
    in_maps, xsqh, wsqh = _shard_inputs(x, W, mm_dtype_name)
    res = bass_utils.run_bass_kernel_spmd(
        nc, in_maps, core_ids=list(range(N_CORES)), trace=trace
    )
    return _gather(res.results, xsqh, wsqh), res


def kernel(x, W):
    out, _ = run(x, W, trace=False, mm_dtype_name="float8e4")
    return out
